# revision 1
# baseline (speedup 1.0000x reference)
"""Trainium2 Bass kernel for nn_AttentionLayer (B=2, S=2048, D=1024, H=16, dh=64).

Sharding: head-parallel across 8 NeuronCores — each core computes the Q/K/V
projections for its 2 heads (column slices of Wq/Wk/Wv), causal attention for
its 4 (batch, head) pairs, then an AllToAll exchanges per-head context so each
core runs the output projection for 1/8 of the tokens.

All matmuls run in float32r (tf32-class PE mode, ~4x fp32 throughput,
rel err ~1e-4). Softmax skips the max-subtraction (|scores| <= 8 after the
1/sqrt(64) scale, since q/k are tanh outputs), so exp is a single ACT pass and
row sums come from an appended ones-column in the alpha @ V matmul.

The AllToAll is split per local head: the h=0 exchange overlaps the h=1
attention compute, and the output projection accumulates each 64-row half as
soon as its exchange lands (K=64 row-packed matmuls).

Self-contained: accepts the full unsharded inputs, returns the full output.
"""

import os

import numpy as np

import concourse.bass as bass
import concourse.mybir as mybir
import concourse.tile as tile
from concourse import bacc
from concourse.bass_utils import run_bass_kernel_spmd

B, S, D = 2, 2048, 1024
H, DH = 16, 64
N_CORES = 8
HPC = H // N_CORES          # heads per core (2)
LC = HPC * DH               # local projection columns (128)
T = B * S                   # total tokens (4096)
TBLK = T // N_CORES         # tokens per output block (512)
NEG = -1.0e9

f32 = mybir.dt.float32
f32r = mybir.dt.float32r

SINGLE_A2A = bool(int(os.environ.get("K_SINGLE_A2A", "0")))
NO_SCALAR_DMA = bool(int(os.environ.get("K_NO_SCALAR_DMA", "0")))

_CACHE = {}
LAST_RESULTS = None


def _build():
    nc = bacc.Bacc("TRN2", target_bir_lowering=False, debug=False,
                   num_devices=N_CORES)

    statesT = nc.dram_tensor("statesT", [D, T], f32r, kind="ExternalInput")
    wq = nc.dram_tensor("wq", [D, LC], f32r, kind="ExternalInput")
    wk = nc.dram_tensor("wk", [D, LC], f32r, kind="ExternalInput")
    wv = nc.dram_tensor("wv", [D, LC], f32r, kind="ExternalInput")
    wo = nc.dram_tensor("wo", [D, D], f32r, kind="ExternalInput")
    bq = nc.dram_tensor("bq", [LC, 1], f32, kind="ExternalInput")
    bk = nc.dram_tensor("bk", [LC, 1], f32, kind="ExternalInput")
    bv = nc.dram_tensor("bv", [LC, 1], f32, kind="ExternalInput")
    bo = nc.dram_tensor("bo", [D, 1], f32, kind="ExternalInput")
    masks = nc.dram_tensor("masks", [128, 4, 512], mybir.dt.bfloat16, kind="ExternalInput")
    ident = nc.dram_tensor("ident", [128, 128], f32r, kind="ExternalInput")
    ones = nc.dram_tensor("ones", [128, 64], f32r, kind="ExternalInput")

    if SINGLE_A2A:
        a2a_in_c = nc.dram_tensor("a2a_in_c", [N_CORES, LC, TBLK], f32r)
        a2a_out_c = nc.dram_tensor("a2a_out_c", [N_CORES, LC, TBLK], f32r)
    else:
        a2a_in = [nc.dram_tensor(f"a2a_in{h}", [N_CORES, DH, TBLK], f32r)
                  for h in range(HPC)]
        a2a_out = [nc.dram_tensor(f"a2a_out{h}", [N_CORES, DH, TBLK], f32r)
                   for h in range(HPC)]
    out = nc.dram_tensor("out", [D, TBLK], f32, kind="ExternalOutput")

    Tanh = mybir.ActivationFunctionType.Tanh
    Exp = mybir.ActivationFunctionType.Exp

    with tile.TileContext(nc) as tc:
        with (
            tc.tile_pool(name="consts", bufs=1) as consts,
            tc.tile_pool(name="persist", bufs=1) as persist,
            tc.tile_pool(name="stream", bufs=4) as stream,
            tc.tile_pool(name="vtp", bufs=2) as vtp,
            tc.tile_pool(name="etp", bufs=7) as etp,
            tc.tile_pool(name="cxp", bufs=2) as cxp,
            tc.tile_pool(name="outp", bufs=3) as outp,
            # one PSUM pool for the whole program: tag "mm" [128,2,512] x3 =
            # 6 banks, tag "acc" [128,512] x2 = 2 banks -> 8 banks total
            tc.tile_pool(name="psum", bufs=1, space="PSUM") as psum,
        ):
            # ---- constants / weights in SBUF ----
            wq_sb = consts.tile([128, 8, LC], f32r)
            wk_sb = consts.tile([128, 8, LC], f32r)
            wv_sb = consts.tile([128, 8, LC], f32r)
            nc.sync.dma_start(out=wq_sb, in_=wq.ap().rearrange("(k p) l -> p k l", p=128))
            nc.sync.dma_start(out=wk_sb, in_=wk.ap().rearrange("(k p) l -> p k l", p=128))
            nc.sync.dma_start(out=wv_sb, in_=wv.ap().rearrange("(k p) l -> p k l", p=128))
            bq_sb = consts.tile([LC, 1], f32)
            bk_sb = consts.tile([LC, 1], f32)
            bv_sb = consts.tile([LC, 1], f32)
            nc.sync.dma_start(out=bq_sb, in_=bq[:, :])
            nc.sync.dma_start(out=bk_sb, in_=bk[:, :])
            nc.sync.dma_start(out=bv_sb, in_=bv[:, :])
            masks_sb = consts.tile([128, 4, 512], mybir.dt.bfloat16)
            nc.sync.dma_start(out=masks_sb, in_=masks[:, :, :])
            ident_sb = consts.tile([128, 128], f32r)
            nc.sync.dma_start(out=ident_sb, in_=ident[:, :])
            ones_sb = consts.tile([128, 64], f32r)
            nc.sync.dma_start(out=ones_sb, in_=ones[:, :])
            ident16 = consts.tile([128, 128], mybir.dt.bfloat16)
            nc.vector.tensor_copy(ident16, ident_sb)
            # wo/bo are only needed by phase 3; keep them on the scalar
            # (ACT) HWDGE queue behind the statesT tiles it also carries.
            wo_sb = persist.tile([128, 8, D], f32r)
            bo_sb = consts.tile([128, 8, 1], f32)

            # ---- phase 1: Q/K/V projections (transposed layout) ----
            # per-head tiles, partitions = (batch*64 + d): lets the two
            # batches' score matmuls row-pack into different PE row groups
            qt_h = [persist.tile([128, S], mybir.dt.bfloat16, name=f"qt_h{i}")
                    for i in range(HPC)]
            kt_h = [persist.tile([128, S], mybir.dt.bfloat16, name=f"kt_h{i}")
                    for i in range(HPC)]
            # v5: per 128-token tile, [tok_local, (h0 V | ones | h1 V | ones)]
            v5_sb = persist.tile([128, T // 128, 2 * (DH + 1)], f32r)
            nc.vector.tensor_copy(
                v5_sb.rearrange("p t (a b) -> p (t a) b", a=2)[:, :, DH:DH + 1].opt(),
                ones_sb[:, :].opt(),
            )

            for tt in range(T // 1024):  # 4 double-width token tiles
                acc_q = psum.tile([128, 2, 512], f32, tag="mm", bufs=3)
                acc_k = psum.tile([128, 2, 512], f32, tag="mm", bufs=3)
                acc_v = psum.tile([128, 2, 512], f32, tag="mm", bufs=3)
                for kk in range(8):
                    st = stream.tile([128, 1024], f32r, tag="st")
                    # early tiles on sync (gpsimd/scalar queues sit behind
                    # the ~30-60us kernel-entry barrier); late odd tiles on
                    # gpsimd SWDGE so the scalar engine issues no DMAs at all
                    dma_eng = nc.sync if (NO_SCALAR_DMA or tt < 2 or kk % 2 == 0) else nc.gpsimd
                    dma_eng.dma_start(
                        out=st,
                        in_=statesT[128 * kk:128 * (kk + 1),
                                    1024 * tt:1024 * (tt + 1)],
                    )
                    for acc, w_sb in ((acc_q, wq_sb), (acc_k, wk_sb), (acc_v, wv_sb)):
                        for half in range(2):
                            nc.tensor.matmul(acc[:, half, :], w_sb[:, kk, :],
                                             st[:, 512 * half:512 * (half + 1)],
                                             start=(kk == 0), stop=(kk == 7))
                bb = tt // 2                     # which batch this tt is in
                sl = slice(1024 * (tt % 2), 1024 * (tt % 2 + 1))
                for hh in range(HPC):            # 32-aligned partition shift
                    hsl = slice(DH * hh, DH * (hh + 1))
                    bsl = slice(DH * bb, DH * (bb + 1))
                    nc.scalar.activation(out=qt_h[hh][bsl, sl], in_=acc_q[hsl, :, :],
                                         func=Tanh, bias=bq_sb[hsl, :])
                    nc.scalar.activation(out=kt_h[hh][bsl, sl], in_=acc_k[hsl, :, :],
                                         func=Tanh, bias=bk_sb[hsl, :])
                vt_c = vtp.tile([128, 1024], f32r, tag="vt")
                nc.scalar.activation(out=vt_c, in_=acc_v, func=Tanh, bias=bv_sb)
                # transpose each 128-col block of vt into v5 (both heads at once)
                for j in range(8):
                    t_idx = 8 * tt + j
                    trp = psum.tile([128, 512], f32r, tag="acc", bufs=2)
                    nc.tensor.transpose(trp[:, 0:128],
                                        vt_c[:, 128 * j:128 * (j + 1)], ident_sb)
                    nc.vector.tensor_copy(
                        v5_sb.rearrange("p t (a b) -> p t a b", a=2)[:, t_idx, :, 0:DH],
                        trp[:, 0:128].rearrange("p (a b) -> p a b", a=2),
                    )

            # ---- phase 2: causal attention, h-outer for split A2A ----
            for h in range(HPC):
                p0 = DH * h
                group = []  # (cl_sb, tb_idx) pending normalization
                def flush_group():
                    # batched 1/l: copy each group's l-row to a distinct
                    # 32-aligned partition, one reciprocal serves them all
                    lb = cxp.tile([128, 512], f32, tag="lb", bufs=1)
                    for i, (cl_sb, _) in enumerate(group):
                        nc.vector.tensor_copy(lb[32 * i:32 * i + 1, :],
                                              cl_sb[DH:DH + 1, :])
                    rbat = cxp.tile([128, 512], f32r, tag="rbat", bufs=1)
                    with nc.allow_low_precision(reason="f32r == f32 storage"):
                        nc.vector.reciprocal(out=rbat, in_=lb)
                    for i, (cl_sb, tb_idx) in enumerate(group):
                        rbp = psum.tile([128, 512], f32, tag="acc", bufs=2)
                        nc.tensor.matmul(rbp[0:DH, :], ones_sb[32 * i:32 * i + 1, :],
                                         rbat[32 * i:32 * i + 1, :],
                                         start=True, stop=True,
                                         tile_position=(32 * i, 0))
                        cx = cxp.tile([DH, 512], f32r, tag="cx")
                        nc.vector.tensor_mul(cx, cl_sb[0:DH, :], rbp[0:DH, :])
                        if SINGLE_A2A:
                            nc.sync.dma_start(
                                out=a2a_in_c[tb_idx, p0:p0 + DH, :], in_=cx)
                        else:
                            nc.sync.dma_start(out=a2a_in[h][tb_idx, :, :], in_=cx)
                    group.clear()

                for qi in range(4):
                    nkt = 4 * qi + 4       # causal kt tiles (128 wide)
                    q_lo = 512 * qi
                    ctxps = [psum.tile([128, 512], f32, tag="acc", bufs=2,
                                       name=f"ctxp_h{h}q{qi}b{b}")
                             for b in range(B)]
                    for ch in range(nkt // 2):
                        for b in range(B):  # the two batches' STs row-pack
                            pb = DH * b
                            diag = ch >= 2 * qi
                            stp = psum.tile([128, 2, 512], f32, tag="mm", bufs=3)
                            for j in range(2):
                                ktj = 2 * ch + j
                                k_lo = 128 * ktj
                                nc.tensor.matmul(
                                    stp[:, j, :],
                                    kt_h[h][pb:pb + DH, k_lo:k_lo + 128],
                                    qt_h[h][pb:pb + DH, q_lo:q_lo + 512],
                                    start=True, stop=not diag,
                                )
                                if diag:  # causal bias via PE accumulate
                                    moff = (ch - 2 * qi) * 2
                                    nc.tensor.matmul(
                                        stp[:, j, :], ident16,
                                        masks_sb[:, moff + j, :],
                                        start=False, stop=True,
                                    )
                            et = etp.tile([128, 2, 512], f32r, tag="et")
                            nc.scalar.activation(out=et, in_=stp, func=Exp,
                                                 scale=0.125)
                            for j in range(2):
                                ktj = 2 * ch + j
                                nc.tensor.matmul(
                                    ctxps[b][0:DH + 1, :],
                                    v5_sb[:, 16 * b + ktj, 65 * h:65 * h + 65],
                                    et[:, j, :],
                                    start=(ktj == 0), stop=(ktj == nkt - 1),
                                )
                    for b in range(B):
                        # copy ctx+l out of PSUM eagerly (PSUM slot recycles)
                        cl_sb = cxp.tile([DH + 1, 512], f32, tag="cl", bufs=5)
                        nc.vector.tensor_copy(cl_sb, ctxps[b][0:DH + 1, :])
                        group.append((cl_sb, 4 * b + qi))
                        if len(group) == 2:  # small groups: shorter serial
                            flush_group()    # chain before the A2A trigger
                assert not group
                if not SINGLE_A2A:
                    # per-head exchange: h=0 overlaps h=1 compute
                    nc.gpsimd.collective_compute(
                        "AllToAll", mybir.AluOpType.bypass,
                        replica_groups=[list(range(N_CORES))],
                        ins=[a2a_in[h][:].opt()], outs=[a2a_out[h][:].opt()],
                    )
            if SINGLE_A2A:
                nc.gpsimd.collective_compute(
                    "AllToAll", mybir.AluOpType.bypass,
                    replica_groups=[list(range(N_CORES))],
                    ins=[a2a_in_c[:].opt()], outs=[a2a_out_c[:].opt()],
                )

            # ---- phase 3: output projection, per-half accumulation ----
            wo_eng = nc.sync if NO_SCALAR_DMA else nc.gpsimd
            wo_eng.dma_start(out=wo_sb,
                             in_=wo.ap().rearrange("(k p) o -> p k o", p=128))
            wo_eng.dma_start(out=bo_sb,
                             in_=bo.ap().rearrange("(k p) one -> p k one", p=128))
            cxt0s, cxt1s = [], []
            for kc in range(8):
                cxt0 = outp.tile([128, 512], f32r, tag="cxt0", bufs=8)
                cxt1 = outp.tile([128, 512], f32r, tag="cxt1", bufs=8)
                if SINGLE_A2A:
                    nc.sync.dma_start(out=cxt0[0:DH, :], in_=a2a_out_c[kc, 0:DH, :])
                    nc.gpsimd.dma_start(out=cxt1[DH:128, :], in_=a2a_out_c[kc, DH:128, :])
                else:
                    nc.sync.dma_start(out=cxt0[0:DH, :], in_=a2a_out[0][kc, :, :])
                    nc.gpsimd.dma_start(out=cxt1[DH:128, :], in_=a2a_out[1][kc, :, :])
                cxt0s.append(cxt0)
                cxt1s.append(cxt1)
            s0s = []
            for oc in range(8):  # h=0 half: runs as soon as A2A#1 lands
                op0 = psum.tile([128, 512], f32, tag="acc", bufs=2)
                osl = slice(128 * oc, 128 * (oc + 1))
                for kc in range(8):
                    nc.tensor.matmul(op0, wo_sb[0:DH, kc, osl], cxt0s[kc][0:DH, :],
                                     start=(kc == 0), stop=(kc == 7))
                s0 = outp.tile([128, 512], f32, tag="s0", bufs=8)
                nc.vector.tensor_copy(s0, op0)
                s0s.append(s0)
            for oc in range(8):  # h=1 half after A2A#2, then combine
                op1 = psum.tile([128, 512], f32, tag="acc", bufs=2)
                osl = slice(128 * oc, 128 * (oc + 1))
                for kc in range(8):
                    nc.tensor.matmul(op1, wo_sb[DH:128, kc, osl],
                                     cxt1s[kc][DH:128, :],
                                     start=(kc == 0), stop=(kc == 7))
                s1 = outp.tile([128, 512], f32, tag="s1", bufs=2)
                nc.vector.tensor_add(s1, s0s[oc], op1)
                osb = outp.tile([128, 512], f32, tag="osb", bufs=2)
                nc.scalar.activation(out=osb, in_=s1, func=Tanh, bias=bo_sb[:, oc, :])
                nc.sync.dma_start(out=out[osl, :], in_=osb)

    nc.compile()
    return nc


def _get_nc():
    if "nc" not in _CACHE:
        _CACHE["nc"] = _build()
    return _CACHE["nc"]


def _make_masks():
    kt_local = np.arange(128)[:, None, None]
    j = np.arange(4)[None, :, None]
    q_local = np.arange(512)[None, None, :]
    import ml_dtypes
    return np.where(q_local >= 128 * j + kt_local, 0.0, NEG).astype(ml_dtypes.bfloat16)


def kernel(states, Wq, bq, Wk, bk, Wv, bv, Wo, bo):
    global LAST_RESULTS
    states = np.asarray(states, dtype=np.float32)
    Wq, Wk, Wv, Wo = (np.asarray(w, dtype=np.float32) for w in (Wq, Wk, Wv, Wo))
    bq, bk, bv, bo = (np.asarray(x, dtype=np.float32) for x in (bq, bk, bv, bo))

    statesT = np.ascontiguousarray(states.reshape(T, D).T)
    masks = _make_masks()
    ident = np.eye(128, dtype=np.float32)
    ones = np.ones((128, 64), dtype=np.float32)

    in_maps = []
    for c in range(N_CORES):
        sl = slice(LC * c, LC * (c + 1))
        in_maps.append({
            "statesT": statesT,
            "wq": np.ascontiguousarray(Wq[:, sl]),
            "wk": np.ascontiguousarray(Wk[:, sl]),
            "wv": np.ascontiguousarray(Wv[:, sl]),
            "wo": Wo,
            "bq": np.ascontiguousarray(bq[sl]).reshape(LC, 1),
            "bk": np.ascontiguousarray(bk[sl]).reshape(LC, 1),
            "bv": np.ascontiguousarray(bv[sl]).reshape(LC, 1),
            "bo": bo.reshape(D, 1),
            "masks": masks,
            "ident": ident,
            "ones": ones,
        })

    nc = _get_nc()
    res = run_bass_kernel_spmd(nc, in_maps, core_ids=list(range(N_CORES)))
    LAST_RESULTS = res

    full = np.empty((T, D), dtype=np.float32)
    for c in range(N_CORES):
        full[TBLK * c:TBLK * (c + 1), :] = res.results[c]["out"].T
    return full.reshape(B, S, D)



# revision 11
# speedup vs baseline: 1.1842x; 1.1842x over previous
"""Trainium2 Bass kernel for nn_AttentionLayer (B=2, S=2048, D=1024, H=16, dh=64).

Sharding: head-parallel across 8 NeuronCores - each core computes the Q/K/V
projections for its 2 heads (column slices of Wq/Wk/Wv), causal attention for
its 4 (batch, head) pairs, then an AllToAll exchanges per-head context so each
core runs the output projection for 1/8 of the tokens.

v3 notes (vs the f32r baseline at 359us):
- bf16 matmul operands everywhere (PE runs 512-free matmuls at ~216ns
  back-to-back regardless of dtype; bf16 halves DMA + SBUF + A2A payload).
- every matmul stationary sits at partition offset 0: per-(head,batch) qt/kt
  tiles of shape [64, S]. Offset-64 stationaries measured 2x slower
  (LDWEIGHTS 326ns vs 97ns, mm 427ns vs 216ns).
- causal mask is a 0/1 bf16 multiply on DVE after the exp, not a PE
  mask-add matmul.
- output projection is one K=128 accumulation chain per 128-column block,
  moving operand from a single [128, 8, 512] tile filled by 2 big DMAs.
- softmax 1/l uses reciprocal_approx_fast (single DVE pass, ~18 bits).
  The custom-DVE op needs its operand at partition 0 (offset-64 input
  produced NaN on hardware), hence the lrow partition-shift copy.
- phase 2 is software-pipelined one (ch,b) slot deep: scores(n), exp(n),
  ctx(n-1); ACT exp (~1.15us per [128,2,512] tile) is the phase-2 floor.
- phase 1 and 2 are interleaved (p1 first halves -> h0 qi0/qi1 -> p1 second
  halves -> rest) so exp starts early and p1's PSUM-recycle bubbles are
  filled with score/ctx matmuls.
- PSUM: "mm" (stp / acc_q / acc_k) 2x[128,2,512] = 4 banks, "acc"
  (ctx pair / acc_v halves) 2 banks, "rbp" (1/l broadcast, transposes,
  outproj) 2 banks.
"""

import os

import numpy as np

import concourse.bass as bass
import concourse.mybir as mybir
import concourse.tile as tile
from concourse import bacc
from concourse.bass_utils import run_bass_kernel_spmd

B, S, D = 2, 2048, 1024
H, DH = 16, 64
N_CORES = 8
HPC = H // N_CORES          # heads per core (2)
LC = HPC * DH               # local projection columns (128)
T = B * S                   # total tokens (4096)
TBLK = T // N_CORES         # tokens per output block (512)

f32 = mybir.dt.float32
bf16 = mybir.dt.bfloat16

_CACHE = {}
LAST_RESULTS = None


def _build():
    nc = bacc.Bacc("TRN2", target_bir_lowering=False, debug=False,
                   num_devices=N_CORES)

    statesT = nc.dram_tensor("statesT", [D, T], bf16, kind="ExternalInput")
    wq = nc.dram_tensor("wq", [D, LC], bf16, kind="ExternalInput")
    wk = nc.dram_tensor("wk", [D, LC], bf16, kind="ExternalInput")
    wv = nc.dram_tensor("wv", [D, LC], bf16, kind="ExternalInput")
    wo = nc.dram_tensor("wo", [D, D], bf16, kind="ExternalInput")
    bq = nc.dram_tensor("bq", [LC, 1], f32, kind="ExternalInput")
    bk = nc.dram_tensor("bk", [LC, 1], f32, kind="ExternalInput")
    bv = nc.dram_tensor("bv", [LC, 1], f32, kind="ExternalInput")
    bo = nc.dram_tensor("bo", [D, 1], f32, kind="ExternalInput")
    # multiplicative causal masks: masks[p, m, q] = (q >= 128*m + p)
    masks = nc.dram_tensor("masks", [128, 4, 512], bf16, kind="ExternalInput")
    ident = nc.dram_tensor("ident", [128, 128], bf16, kind="ExternalInput")
    ones_r = nc.dram_tensor("ones_r", [128, 64], bf16, kind="ExternalInput")

    a2a_in = [nc.dram_tensor(f"a2a_in{h}", [N_CORES, DH, TBLK], bf16)
              for h in range(HPC)]
    a2a_out = [nc.dram_tensor(f"a2a_out{h}", [N_CORES, DH, TBLK], bf16)
               for h in range(HPC)]
    out = nc.dram_tensor("out", [D, TBLK], f32, kind="ExternalOutput")

    Tanh = mybir.ActivationFunctionType.Tanh
    Exp = mybir.ActivationFunctionType.Exp

    with tile.TileContext(nc) as tc:
        with (
            tc.tile_pool(name="consts", bufs=1) as consts,
            tc.tile_pool(name="persist", bufs=1) as persist,
            tc.tile_pool(name="stream", bufs=4) as stream,
            tc.tile_pool(name="vtp", bufs=2) as vtp,
            tc.tile_pool(name="etp", bufs=6) as etp,
            tc.tile_pool(name="cxp", bufs=2) as cxp,
            tc.tile_pool(name="outp", bufs=2) as outp,
            tc.tile_pool(name="psum", bufs=1, space="PSUM") as psum,
        ):
            # ---- constants / weights ----
            # sync queue: projection weights then the statesT stream (the
            # only queue that is live from kernel entry). Everything not
            # needed in the first ~20us goes on the gpsimd queue.
            wq_sb = consts.tile([128, 8, LC], bf16)
            wk_sb = consts.tile([128, 8, LC], bf16)
            wv_sb = consts.tile([128, 8, LC], bf16)
            nc.sync.dma_start(out=wq_sb, in_=wq.ap().rearrange("(k p) l -> p k l", p=128))
            nc.sync.dma_start(out=wk_sb, in_=wk.ap().rearrange("(k p) l -> p k l", p=128))
            nc.sync.dma_start(out=wv_sb, in_=wv.ap().rearrange("(k p) l -> p k l", p=128))
            bq_sb = consts.tile([LC, 1], f32)
            bk_sb = consts.tile([LC, 1], f32)
            bv_sb = consts.tile([LC, 1], f32)
            nc.gpsimd.dma_start(out=bq_sb, in_=bq[:, :])
            nc.gpsimd.dma_start(out=bk_sb, in_=bk[:, :])
            nc.gpsimd.dma_start(out=bv_sb, in_=bv[:, :])
            ident16 = consts.tile([128, 128], bf16)
            nc.gpsimd.dma_start(out=ident16, in_=ident[:, :])
            ones_sb = consts.tile([128, 64], bf16)
            nc.gpsimd.dma_start(out=ones_sb, in_=ones_r[:, :])
            masks_sb = consts.tile([128, 4, 512], bf16)
            nc.gpsimd.dma_start(out=masks_sb, in_=masks[:, :, :])
            wo_sb = persist.tile([128, 8, D], bf16)
            bo_sb = consts.tile([128, 8, 1], f32)
            nc.gpsimd.dma_start(out=wo_sb,
                                in_=wo.ap().rearrange("(k p) o -> p k o", p=128))
            nc.gpsimd.dma_start(out=bo_sb,
                                in_=bo.ap().rearrange("(k p) one -> p k one", p=128))

            # ---- phase 1: Q/K/V projections (transposed layout) ----
            qt_hb = [[persist.tile([DH, S], bf16, name=f"qt_h{h}b{b}")
                      for b in range(B)] for h in range(HPC)]
            kt_hb = [[persist.tile([DH, S], bf16, name=f"kt_h{h}b{b}")
                      for b in range(B)] for h in range(HPC)]
            # v5: per 128-token tile, [tok_local, head, (V cols | ones)]
            v5_sb = persist.tile([128, T // 128, HPC, DH + 1], bf16)
            nc.vector.tensor_copy(v5_sb[:, :, :, DH:DH + 1].opt(), ones_sb[:, :].opt())

            def emit_p1(tt):
                acc_q = psum.tile([128, 2, 512], f32, tag="mm", bufs=2,
                                  name=f"acc_q{tt}")
                acc_k = psum.tile([128, 2, 512], f32, tag="mm", bufs=2,
                                  name=f"acc_k{tt}")
                acc_v = [psum.tile([128, 512], f32, tag="acc", bufs=2,
                                   name=f"acc_v{tt}_{half}") for half in range(2)]
                for kk in range(8):
                    st = stream.tile([128, 1024], bf16, tag="st")
                    nc.sync.dma_start(
                        out=st,
                        in_=statesT[128 * kk:128 * (kk + 1),
                                    1024 * tt:1024 * (tt + 1)],
                    )
                    for half in range(2):
                        mv = st[:, 512 * half:512 * (half + 1)]
                        nc.tensor.matmul(acc_q[:, half, :], wq_sb[:, kk, :], mv,
                                         start=(kk == 0), stop=(kk == 7))
                        nc.tensor.matmul(acc_k[:, half, :], wk_sb[:, kk, :], mv,
                                         start=(kk == 0), stop=(kk == 7))
                        nc.tensor.matmul(acc_v[half], wv_sb[:, kk, :], mv,
                                         start=(kk == 0), stop=(kk == 7))
                bb = tt // 2                     # which batch this tt is in
                sl = slice(1024 * (tt % 2), 1024 * (tt % 2 + 1))
                for hh in range(HPC):
                    hsl = slice(DH * hh, DH * (hh + 1))
                    nc.scalar.activation(out=qt_hb[hh][bb][:, sl], in_=acc_q[hsl, :, :],
                                         func=Tanh, bias=bq_sb[hsl, :])
                    nc.scalar.activation(out=kt_hb[hh][bb][:, sl], in_=acc_k[hsl, :, :],
                                         func=Tanh, bias=bk_sb[hsl, :])
                vt_c = vtp.tile([128, 1024], bf16, tag="vt")
                for half in range(2):
                    nc.scalar.activation(out=vt_c[:, 512 * half:512 * (half + 1)],
                                         in_=acc_v[half], func=Tanh, bias=bv_sb)
                # transpose each 128-col block of vt into v5 (both heads at once)
                for j in range(8):
                    t_idx = 8 * tt + j
                    trp = psum.tile([128, 1024], bf16, tag="rbp", bufs=2,
                                    name=f"trp{tt}_{j}")
                    nc.tensor.transpose(trp[:, 0:128],
                                        vt_c[:, 128 * j:128 * (j + 1)], ident16)
                    nc.vector.tensor_copy(
                        v5_sb[:, t_idx, :, 0:DH],
                        trp[:, 0:128].rearrange("p (a b) -> p a b", a=2),
                    )

            # ---- phase 2 machinery ----
            def make_p2_state():
                return {"ctxp": {}, "prev": None}

            def emit_ctx(h, st, slot, et_t):
                qi, ch, b = slot
                nkt = 4 * qi + 4
                if ch == 0:
                    st["ctxp"][(qi, b)] = psum.tile(
                        [DH + 1, 512], f32, tag="acc", bufs=2,
                        name=f"ctxp_h{h}q{qi}b{b}")
                cp = st["ctxp"][(qi, b)]
                for j in range(2):
                    ktj = 2 * ch + j
                    nc.tensor.matmul(
                        cp,
                        v5_sb[:, 16 * b + ktj, h, :],
                        et_t[:, j, :],
                        start=(ktj == 0), stop=(ktj == nkt - 1),
                    )
                if ch == 2 * qi + 1:  # ctx for (qi,b) complete
                    finish_member(h, qi, b, cp)

            def finish_member(h, qi, b, cp):
                # copy ctx+l out of PSUM (slot recycles), normalize, ship
                cl_sb = cxp.tile([DH + 1, 512], f32, tag="cl", bufs=6)
                nc.vector.tensor_copy(cl_sb, cp)
                # custom-DVE recip needs its operand at partition 0
                lrow = cxp.tile([1, 512], f32, tag="lrow", bufs=2)
                nc.vector.tensor_copy(lrow, cl_sb[DH:DH + 1, :])
                rb = cxp.tile([1, 512], f32, tag="rb", bufs=2)
                with nc.allow_low_precision(reason="softmax 1/l, ~18 bits"):
                    nc.vector.reciprocal_approx_fast(out=rb, in_=lrow)
                # custom-DVE f32 out can't legally feed an f32r matmul;
                # broadcast 1/l through the PE in bf16 instead
                rb16 = cxp.tile([1, 512], bf16, tag="rb16", bufs=2)
                with nc.allow_low_precision(reason="bf16 1/l broadcast"):
                    nc.vector.tensor_copy(rb16, rb)
                rbp = psum.tile([DH, 512], f32, tag="rbp", bufs=2, name="rbp")
                nc.tensor.matmul(rbp, ones_sb[0:1, :], rb16,
                                 start=True, stop=True)
                cx = cxp.tile([DH, 512], bf16, tag="cx", bufs=2)
                with nc.allow_low_precision(reason="bf16 context"):
                    nc.vector.tensor_mul(cx, cl_sb[0:DH, :], rbp)
                nc.sync.dma_start(out=a2a_in[h][4 * b + qi, :, :], in_=cx)

            def emit_p2(h, qis, st):
                slots = [(qi, ch, b)
                         for qi in qis
                         for ch in range(2 * qi + 2)
                         for b in range(B)]
                for slot in slots:
                    qi, ch, b = slot
                    stp = psum.tile([128, 2, 512], f32, tag="mm", bufs=2,
                                    name=f"stp_h{h}q{qi}c{ch}b{b}")
                    q_lo = 512 * qi
                    for j in range(2):
                        k_lo = 128 * (2 * ch + j)
                        nc.tensor.matmul(
                            stp[:, j, :],
                            kt_hb[h][b][:, k_lo:k_lo + 128],
                            qt_hb[h][b][:, q_lo:q_lo + 512],
                            start=True, stop=True,
                        )
                    et_t = etp.tile([128, 2, 512], bf16, tag="et")
                    nc.scalar.activation(out=et_t, in_=stp, func=Exp, scale=0.125)
                    if ch >= 2 * qi:  # diagonal: 0/1 mask multiply on DVE
                        moff = (ch - 2 * qi) * 2
                        with nc.allow_low_precision(reason="0/1 mask"):
                            nc.vector.tensor_mul(
                                et_t, et_t, masks_sb[:, moff:moff + 2, :])
                    if st["prev"] is not None:
                        emit_ctx(h, st, st["prev"][0], st["prev"][1])
                    st["prev"] = (slot, et_t)

            def drain_p2(h, st):
                if st["prev"] is not None:
                    emit_ctx(h, st, st["prev"][0], st["prev"][1])
                    st["prev"] = None

            # ---- interleaved schedule ----
            st0 = make_p2_state()
            st1 = make_p2_state()
            emit_p1(0)
            emit_p1(2)
            emit_p2(0, [0, 1], st0)     # h0 first half (needs tokens < 1024)
            emit_p1(1)
            emit_p1(3)
            emit_p2(0, [2, 3], st0)
            drain_p2(0, st0)
            nc.gpsimd.collective_compute(
                "AllToAll", mybir.AluOpType.bypass,
                replica_groups=[list(range(N_CORES))],
                ins=[a2a_in[0][:].opt()], outs=[a2a_out[0][:].opt()],
            )
            emit_p2(1, [0, 1, 2, 3], st1)
            drain_p2(1, st1)
            nc.gpsimd.collective_compute(
                "AllToAll", mybir.AluOpType.bypass,
                replica_groups=[list(range(N_CORES))],
                ins=[a2a_in[1][:].opt()], outs=[a2a_out[1][:].opt()],
            )

            # ---- phase 3: output projection, single K=128 chain ----
            cxt_all = outp.tile([128, 8, 512], bf16, tag="cxt", bufs=1)
            nc.sync.dma_start(out=cxt_all[0:DH, :, :],
                              in_=a2a_out[0].ap().rearrange("k d t -> d k t"))
            nc.gpsimd.dma_start(out=cxt_all[DH:128, :, :],
                                in_=a2a_out[1].ap().rearrange("k d t -> d k t"))
            for oc in range(8):
                op = psum.tile([128, 512], f32, tag=("rbp" if oc % 2 else "acc"),
                               bufs=2, name=f"op{oc}")
                osl = slice(128 * oc, 128 * (oc + 1))
                for kc in range(8):
                    nc.tensor.matmul(op, wo_sb[:, kc, osl], cxt_all[:, kc, :],
                                     start=(kc == 0), stop=(kc == 7))
                osb = outp.tile([128, 512], f32, tag="osb", bufs=2)
                nc.scalar.activation(out=osb, in_=op, func=Tanh, bias=bo_sb[:, oc, :])
                nc.sync.dma_start(out=out[osl, :], in_=osb)

    nc.compile()
    return nc


def _get_nc():
    if "nc" not in _CACHE:
        _CACHE["nc"] = _build()
    return _CACHE["nc"]


def _make_masks():
    import ml_dtypes
    kt_local = np.arange(128)[:, None, None]
    j = np.arange(4)[None, :, None]
    q_local = np.arange(512)[None, None, :]
    return (q_local >= 128 * j + kt_local).astype(ml_dtypes.bfloat16)


def kernel(states, Wq, bq, Wk, bk, Wv, bv, Wo, bo):
    global LAST_RESULTS
    import ml_dtypes

    states = np.asarray(states, dtype=np.float32)
    Wq, Wk, Wv, Wo = (np.asarray(w, dtype=np.float32) for w in (Wq, Wk, Wv, Wo))
    bq, bk, bv, bo = (np.asarray(x, dtype=np.float32) for x in (bq, bk, bv, bo))

    statesT = np.ascontiguousarray(states.reshape(T, D).T).astype(ml_dtypes.bfloat16)
    Wo16 = Wo.astype(ml_dtypes.bfloat16)
    masks = _make_masks()
    ident = np.eye(128, dtype=np.float32).astype(ml_dtypes.bfloat16)
    ones_r = np.ones((128, 64), dtype=np.float32).astype(ml_dtypes.bfloat16)

    in_maps = []
    for c in range(N_CORES):
        sl = slice(LC * c, LC * (c + 1))
        in_maps.append({
            "statesT": statesT,
            "wq": np.ascontiguousarray(Wq[:, sl]).astype(ml_dtypes.bfloat16),
            "wk": np.ascontiguousarray(Wk[:, sl]).astype(ml_dtypes.bfloat16),
            "wv": np.ascontiguousarray(Wv[:, sl]).astype(ml_dtypes.bfloat16),
            "wo": Wo16,
            "bq": np.ascontiguousarray(bq[sl]).reshape(LC, 1),
            "bk": np.ascontiguousarray(bk[sl]).reshape(LC, 1),
            "bv": np.ascontiguousarray(bv[sl]).reshape(LC, 1),
            "bo": bo.reshape(D, 1),
            "masks": masks,
            "ident": ident,
            "ones_r": ones_r,
        })

    nc = _get_nc()
    res = run_bass_kernel_spmd(nc, in_maps, core_ids=list(range(N_CORES)))
    LAST_RESULTS = res

    full = np.empty((T, D), dtype=np.float32)
    for c in range(N_CORES):
        full[TBLK * c:TBLK * (c + 1), :] = res.results[c]["out"].T
    return full.reshape(B, S, D)


# revision 12
# speedup vs baseline: 1.2742x; 1.0760x over previous
"""Trainium2 Bass kernel for nn_AttentionLayer (B=2, S=2048, D=1024, H=16, dh=64).

Sharding: head-parallel across 8 NeuronCores - each core computes the Q/K/V
projections for its 2 heads (column slices of Wq/Wk/Wv), causal attention for
its 4 (batch, head) pairs, then an AllToAll exchanges per-head context so each
core runs the output projection for 1/8 of the tokens.

v3 notes (vs the f32r baseline at 359us):
- bf16 matmul operands everywhere (PE runs 512-free matmuls at ~216ns
  back-to-back regardless of dtype; bf16 halves DMA + SBUF + A2A payload).
- every matmul stationary sits at partition offset 0: per-(head,batch) qt/kt
  tiles of shape [64, S]. Offset-64 stationaries measured 2x slower
  (LDWEIGHTS 326ns vs 97ns, mm 427ns vs 216ns).
- causal mask is a 0/1 bf16 multiply on DVE after the exp, not a PE
  mask-add matmul.
- output projection is one K=128 accumulation chain per 128-column block,
  moving operand from a single [128, 8, 512] tile filled by 2 big DMAs.
- softmax 1/l uses reciprocal_approx_fast (single DVE pass, ~18 bits).
  The custom-DVE op needs its operand at partition 0 (offset-64 input
  produced NaN on hardware), hence the lrow partition-shift copy.
- phase 2 is software-pipelined one (ch,b) slot deep: scores(n), exp(n),
  ctx(n-1); ACT exp (~1.15us per [128,2,512] tile) is the phase-2 floor.
- phase 1 and 2 are interleaved (p1 first halves -> h0 qi0/qi1 -> p1 second
  halves -> rest) so exp starts early and p1's PSUM-recycle bubbles are
  filled with score/ctx matmuls.
- PSUM: "mm" (stp / acc_q / acc_k) 2x[128,2,512] = 4 banks, "acc"
  (ctx pair / acc_v halves) 2 banks, "rbp" (1/l broadcast, transposes,
  outproj) 2 banks.
"""

import os

import numpy as np

import concourse.bass as bass
import concourse.mybir as mybir
import concourse.tile as tile
from concourse import bacc
from concourse.bass_utils import run_bass_kernel_spmd

B, S, D = 2, 2048, 1024
H, DH = 16, 64
N_CORES = 8
HPC = H // N_CORES          # heads per core (2)
LC = HPC * DH               # local projection columns (128)
T = B * S                   # total tokens (4096)
TBLK = T // N_CORES         # tokens per output block (512)

f32 = mybir.dt.float32
bf16 = mybir.dt.bfloat16

_CACHE = {}
LAST_RESULTS = None


def _build():
    nc = bacc.Bacc("TRN2", target_bir_lowering=False, debug=False,
                   num_devices=N_CORES)

    statesT = nc.dram_tensor("statesT", [D, T], bf16, kind="ExternalInput")
    wq = nc.dram_tensor("wq", [D, LC], bf16, kind="ExternalInput")
    wk = nc.dram_tensor("wk", [D, LC], bf16, kind="ExternalInput")
    wv = nc.dram_tensor("wv", [D, LC], bf16, kind="ExternalInput")
    wo = nc.dram_tensor("wo", [D, D], bf16, kind="ExternalInput")
    bq = nc.dram_tensor("bq", [LC, 1], f32, kind="ExternalInput")
    bk = nc.dram_tensor("bk", [LC, 1], f32, kind="ExternalInput")
    bv = nc.dram_tensor("bv", [LC, 1], f32, kind="ExternalInput")
    bo = nc.dram_tensor("bo", [D, 1], f32, kind="ExternalInput")
    # multiplicative causal masks: masks[p, m, q] = (q >= 128*m + p)
    masks = nc.dram_tensor("masks", [128, 4, 512], bf16, kind="ExternalInput")
    ident = nc.dram_tensor("ident", [128, 128], bf16, kind="ExternalInput")
    ones_r = nc.dram_tensor("ones_r", [128, 64], bf16, kind="ExternalInput")

    a2a_in = [nc.dram_tensor(f"a2a_in{h}", [N_CORES, DH, TBLK], bf16)
              for h in range(HPC)]
    a2a_out = [nc.dram_tensor(f"a2a_out{h}", [N_CORES, DH, TBLK], bf16)
               for h in range(HPC)]
    out = nc.dram_tensor("out", [D, TBLK], f32, kind="ExternalOutput")

    Tanh = mybir.ActivationFunctionType.Tanh
    Exp = mybir.ActivationFunctionType.Exp

    with tile.TileContext(nc) as tc:
        with (
            tc.tile_pool(name="consts", bufs=1) as consts,
            tc.tile_pool(name="persist", bufs=1) as persist,
            tc.tile_pool(name="stream", bufs=4) as stream,
            tc.tile_pool(name="vtp", bufs=2) as vtp,
            tc.tile_pool(name="etp", bufs=6) as etp,
            tc.tile_pool(name="cxp", bufs=2) as cxp,
            tc.tile_pool(name="outp", bufs=2) as outp,
            tc.tile_pool(name="psum", bufs=1, space="PSUM") as psum,
        ):
            # ---- constants / weights ----
            # sync queue: projection weights then the statesT stream (the
            # only queue that is live from kernel entry). Everything not
            # needed in the first ~20us goes on the gpsimd queue.
            wq_sb = consts.tile([128, 8, LC], bf16)
            wk_sb = consts.tile([128, 8, LC], bf16)
            wv_sb = consts.tile([128, 8, LC], bf16)
            nc.sync.dma_start(out=wq_sb, in_=wq.ap().rearrange("(k p) l -> p k l", p=128))
            nc.sync.dma_start(out=wk_sb, in_=wk.ap().rearrange("(k p) l -> p k l", p=128))
            nc.sync.dma_start(out=wv_sb, in_=wv.ap().rearrange("(k p) l -> p k l", p=128))
            bq_sb = consts.tile([LC, 1], f32)
            bk_sb = consts.tile([LC, 1], f32)
            bv_sb = consts.tile([LC, 1], f32)
            nc.gpsimd.dma_start(out=bq_sb, in_=bq[:, :])
            nc.gpsimd.dma_start(out=bk_sb, in_=bk[:, :])
            nc.gpsimd.dma_start(out=bv_sb, in_=bv[:, :])
            ident16 = consts.tile([128, 128], bf16)
            nc.gpsimd.dma_start(out=ident16, in_=ident[:, :])
            ones_sb = consts.tile([128, 64], bf16)
            nc.gpsimd.dma_start(out=ones_sb, in_=ones_r[:, :])
            masks_sb = consts.tile([128, 4, 512], bf16)
            nc.gpsimd.dma_start(out=masks_sb, in_=masks[:, :, :])
            wo_sb = persist.tile([128, 8, D], bf16)
            bo_sb = consts.tile([128, 8, 1], f32)
            nc.gpsimd.dma_start(out=wo_sb,
                                in_=wo.ap().rearrange("(k p) o -> p k o", p=128))
            nc.gpsimd.dma_start(out=bo_sb,
                                in_=bo.ap().rearrange("(k p) one -> p k one", p=128))

            # ---- phase 1: Q/K/V projections (transposed layout) ----
            qt_hb = [[persist.tile([DH, S], bf16, name=f"qt_h{h}b{b}")
                      for b in range(B)] for h in range(HPC)]
            kt_hb = [[persist.tile([DH, S], bf16, name=f"kt_h{h}b{b}")
                      for b in range(B)] for h in range(HPC)]
            # v5: per 128-token tile, [tok_local, head, (V cols | ones)]
            v5_sb = persist.tile([128, T // 128, HPC, DH + 1], bf16)
            nc.vector.tensor_copy(v5_sb[:, :, :, DH:DH + 1].opt(), ones_sb[:, :].opt())

            def emit_p1(tt):
                acc_q = psum.tile([128, 2, 512], f32, tag="mm", bufs=3,
                                  name=f"acc_q{tt}")
                acc_k = psum.tile([128, 2, 512], f32, tag="mm", bufs=3,
                                  name=f"acc_k{tt}")
                acc_v = [psum.tile([128, 512], f32, tag="acc", bufs=2,
                                   name=f"acc_v{tt}_{half}") for half in range(2)]
                for kk in range(8):
                    st = stream.tile([128, 1024], bf16, tag="st")
                    nc.sync.dma_start(
                        out=st,
                        in_=statesT[128 * kk:128 * (kk + 1),
                                    1024 * tt:1024 * (tt + 1)],
                    )
                    for half in range(2):
                        mv = st[:, 512 * half:512 * (half + 1)]
                        nc.tensor.matmul(acc_q[:, half, :], wq_sb[:, kk, :], mv,
                                         start=(kk == 0), stop=(kk == 7))
                        nc.tensor.matmul(acc_k[:, half, :], wk_sb[:, kk, :], mv,
                                         start=(kk == 0), stop=(kk == 7))
                        nc.tensor.matmul(acc_v[half], wv_sb[:, kk, :], mv,
                                         start=(kk == 0), stop=(kk == 7))
                bb = tt // 2                     # which batch this tt is in
                sl = slice(1024 * (tt % 2), 1024 * (tt % 2 + 1))
                for hh in range(HPC):
                    hsl = slice(DH * hh, DH * (hh + 1))
                    nc.scalar.activation(out=qt_hb[hh][bb][:, sl], in_=acc_q[hsl, :, :],
                                         func=Tanh, bias=bq_sb[hsl, :])
                    nc.scalar.activation(out=kt_hb[hh][bb][:, sl], in_=acc_k[hsl, :, :],
                                         func=Tanh, bias=bk_sb[hsl, :])
                vt_c = vtp.tile([128, 1024], bf16, tag="vt")
                for half in range(2):
                    nc.scalar.activation(out=vt_c[:, 512 * half:512 * (half + 1)],
                                         in_=acc_v[half], func=Tanh, bias=bv_sb)
                # transpose each 128-col block of vt into v5 (both heads at once)
                for j in range(8):
                    t_idx = 8 * tt + j
                    trp = psum.tile([128, 1024], bf16, tag="acc", bufs=2,
                                    name=f"trp{tt}_{j}")
                    nc.tensor.transpose(trp[:, 0:128],
                                        vt_c[:, 128 * j:128 * (j + 1)], ident16)
                    nc.vector.tensor_copy(
                        v5_sb[:, t_idx, :, 0:DH],
                        trp[:, 0:128].rearrange("p (a b) -> p a b", a=2),
                    )

            # ---- phase 2 machinery ----
            def make_p2_state():
                return {"ctxp": {}, "prev": None}

            def emit_ctx(h, st, slot, et_t):
                qi, ch, b = slot
                nkt = 4 * qi + 4
                if ch == 0:
                    st["ctxp"][(qi, b)] = psum.tile(
                        [DH + 1, 512], f32, tag="acc", bufs=2,
                        name=f"ctxp_h{h}q{qi}b{b}")
                cp = st["ctxp"][(qi, b)]
                for j in range(2):
                    ktj = 2 * ch + j
                    nc.tensor.matmul(
                        cp,
                        v5_sb[:, 16 * b + ktj, h, :],
                        et_t[:, j, :],
                        start=(ktj == 0), stop=(ktj == nkt - 1),
                    )
                if ch == 2 * qi + 1:  # ctx for (qi,b) complete
                    finish_member(h, qi, b, cp)

            def finish_member(h, qi, b, cp):
                # copy ctx+l out of PSUM (slot recycles), normalize, ship
                cl_sb = cxp.tile([DH + 1, 512], f32, tag="cl", bufs=6)
                nc.vector.tensor_copy(cl_sb, cp)
                # custom-DVE recip needs its operand at partition 0
                lrow = cxp.tile([1, 512], f32, tag="lrow", bufs=2)
                nc.vector.tensor_copy(lrow, cl_sb[DH:DH + 1, :])
                rb = cxp.tile([1, 512], f32, tag="rb", bufs=2)
                with nc.allow_low_precision(reason="softmax 1/l, ~18 bits"):
                    nc.vector.reciprocal_approx_fast(out=rb, in_=lrow)
                # custom-DVE f32 out can't legally feed an f32r matmul;
                # broadcast 1/l through the PE in bf16 instead
                rb16 = cxp.tile([1, 512], bf16, tag="rb16", bufs=2)
                with nc.allow_low_precision(reason="bf16 1/l broadcast"):
                    nc.vector.tensor_copy(rb16, rb)
                rbp = psum.tile([DH, 512], f32, tag="acc", bufs=2, name="rbp")
                nc.tensor.matmul(rbp, ones_sb[0:1, :], rb16,
                                 start=True, stop=True)
                cx = cxp.tile([DH, 512], bf16, tag="cx", bufs=2)
                with nc.allow_low_precision(reason="bf16 context"):
                    nc.vector.tensor_mul(cx, cl_sb[0:DH, :], rbp)
                nc.sync.dma_start(out=a2a_in[h][4 * b + qi, :, :], in_=cx)

            def emit_p2(h, qis, st):
                slots = [(qi, ch, b)
                         for qi in qis
                         for ch in range(2 * qi + 2)
                         for b in range(B)]
                for slot in slots:
                    qi, ch, b = slot
                    stp = psum.tile([128, 2, 512], f32, tag="mm", bufs=3,
                                    name=f"stp_h{h}q{qi}c{ch}b{b}")
                    q_lo = 512 * qi
                    for j in range(2):
                        k_lo = 128 * (2 * ch + j)
                        nc.tensor.matmul(
                            stp[:, j, :],
                            kt_hb[h][b][:, k_lo:k_lo + 128],
                            qt_hb[h][b][:, q_lo:q_lo + 512],
                            start=True, stop=True,
                        )
                    et_t = etp.tile([128, 2, 512], bf16, tag="et")
                    nc.scalar.activation(out=et_t, in_=stp, func=Exp, scale=0.125)
                    if ch >= 2 * qi:  # diagonal: 0/1 mask multiply on DVE
                        moff = (ch - 2 * qi) * 2
                        with nc.allow_low_precision(reason="0/1 mask"):
                            nc.vector.tensor_mul(
                                et_t, et_t, masks_sb[:, moff:moff + 2, :])
                    if st["prev"] is not None:
                        emit_ctx(h, st, st["prev"][0], st["prev"][1])
                    st["prev"] = (slot, et_t)

            def drain_p2(h, st):
                if st["prev"] is not None:
                    emit_ctx(h, st, st["prev"][0], st["prev"][1])
                    st["prev"] = None

            # ---- interleaved schedule ----
            st0 = make_p2_state()
            st1 = make_p2_state()
            emit_p1(0)
            emit_p1(2)
            emit_p2(0, [0, 1], st0)     # h0 first half (needs tokens < 1024)
            emit_p1(1)
            emit_p1(3)
            emit_p2(0, [2, 3], st0)
            drain_p2(0, st0)
            nc.gpsimd.collective_compute(
                "AllToAll", mybir.AluOpType.bypass,
                replica_groups=[list(range(N_CORES))],
                ins=[a2a_in[0][:].opt()], outs=[a2a_out[0][:].opt()],
            )
            emit_p2(1, [0, 1, 2, 3], st1)
            drain_p2(1, st1)
            nc.gpsimd.collective_compute(
                "AllToAll", mybir.AluOpType.bypass,
                replica_groups=[list(range(N_CORES))],
                ins=[a2a_in[1][:].opt()], outs=[a2a_out[1][:].opt()],
            )

            # ---- phase 3: output projection, single K=128 chain ----
            cxt_all = outp.tile([128, 8, 512], bf16, tag="cxt", bufs=1)
            nc.sync.dma_start(out=cxt_all[0:DH, :, :],
                              in_=a2a_out[0].ap().rearrange("k d t -> d k t"))
            nc.sync.dma_start(out=cxt_all[DH:128, :, :],
                              in_=a2a_out[1].ap().rearrange("k d t -> d k t"))
            for oc in range(8):
                op = psum.tile([128, 512], f32, tag="acc", bufs=2, name=f"op{oc}")
                osl = slice(128 * oc, 128 * (oc + 1))
                for kc in range(8):
                    nc.tensor.matmul(op, wo_sb[:, kc, osl], cxt_all[:, kc, :],
                                     start=(kc == 0), stop=(kc == 7))
                osb = outp.tile([128, 512], f32, tag="osb", bufs=2)
                nc.scalar.activation(out=osb, in_=op, func=Tanh, bias=bo_sb[:, oc, :])
                nc.sync.dma_start(out=out[osl, :], in_=osb)

    nc.compile()
    return nc


def _get_nc():
    if "nc" not in _CACHE:
        _CACHE["nc"] = _build()
    return _CACHE["nc"]


def _make_masks():
    import ml_dtypes
    kt_local = np.arange(128)[:, None, None]
    j = np.arange(4)[None, :, None]
    q_local = np.arange(512)[None, None, :]
    return (q_local >= 128 * j + kt_local).astype(ml_dtypes.bfloat16)


def kernel(states, Wq, bq, Wk, bk, Wv, bv, Wo, bo):
    global LAST_RESULTS
    import ml_dtypes

    states = np.asarray(states, dtype=np.float32)
    Wq, Wk, Wv, Wo = (np.asarray(w, dtype=np.float32) for w in (Wq, Wk, Wv, Wo))
    bq, bk, bv, bo = (np.asarray(x, dtype=np.float32) for x in (bq, bk, bv, bo))

    statesT = np.ascontiguousarray(states.reshape(T, D).T).astype(ml_dtypes.bfloat16)
    Wo16 = Wo.astype(ml_dtypes.bfloat16)
    masks = _make_masks()
    ident = np.eye(128, dtype=np.float32).astype(ml_dtypes.bfloat16)
    ones_r = np.ones((128, 64), dtype=np.float32).astype(ml_dtypes.bfloat16)

    in_maps = []
    for c in range(N_CORES):
        sl = slice(LC * c, LC * (c + 1))
        in_maps.append({
            "statesT": statesT,
            "wq": np.ascontiguousarray(Wq[:, sl]).astype(ml_dtypes.bfloat16),
            "wk": np.ascontiguousarray(Wk[:, sl]).astype(ml_dtypes.bfloat16),
            "wv": np.ascontiguousarray(Wv[:, sl]).astype(ml_dtypes.bfloat16),
            "wo": Wo16,
            "bq": np.ascontiguousarray(bq[sl]).reshape(LC, 1),
            "bk": np.ascontiguousarray(bk[sl]).reshape(LC, 1),
            "bv": np.ascontiguousarray(bv[sl]).reshape(LC, 1),
            "bo": bo.reshape(D, 1),
            "masks": masks,
            "ident": ident,
            "ones_r": ones_r,
        })

    nc = _get_nc()
    res = run_bass_kernel_spmd(nc, in_maps, core_ids=list(range(N_CORES)))
    LAST_RESULTS = res

    full = np.empty((T, D), dtype=np.float32)
    for c in range(N_CORES):
        full[TBLK * c:TBLK * (c + 1), :] = res.results[c]["out"].T
    return full.reshape(B, S, D)
